# revision 1
# baseline (speedup 1.0000x reference)
"""MoE arg-classifier (nn_ArgClassifierLayer) on 8 Trainium2 NeuronCores.

Strategy (pure data-parallel over batch, no collectives):
  - The host sorts samples by expert id and deals them to the 8 cores so
    every core holds an identical per-expert segment layout (capacity
    ceil(count/8) per expert, zero-padded). Segment boundaries become
    compile-time constants, so the per-sample grouped GEMM turns into a few
    dense GEMMs over contiguous column ranges.
  - All activations live feature-major ([feat, row]) on chip; the host
    pre-transposes inputs and post-transposes the output, so the device
    never transposes anything.
  - concat([arg, ev]) @ Wm is split into arg @ Wm_top (per row) +
    ev @ Wm_bot (per sample, broadcast over the 28 entity rows with a
    stride-0 access pattern), halving the dominant GEMM's FLOPs.
  - bf16 matmul inputs with fp32 PSUM accumulation; fp32 biases/output.
"""

import math

import numpy as np
import ml_dtypes

import concourse.bass as bass
import concourse.tile as tile
from concourse import bacc, mybir
from concourse.bass_utils import run_bass_kernel_spmd

BF16 = mybir.dt.bfloat16
F32 = mybir.dt.float32
NPBF16 = ml_dtypes.bfloat16

B, E, D = 4096, 28, 500
M, H, R_OUT = 500, 64, 16
NEXP = 34
NCORES = 8
DP = 512  # padded feature dim (both D and M pad to 512)
KT = 4  # k-tiles of 128
P = 128
SC = 18  # samples per compute chunk -> 504 cols <= one PSUM bank
CC = SC * E  # 504

LAST_INFO = {}


def plan_from_evt(evt):
    evt = np.asarray(evt).astype(np.int64)
    counts = np.bincount(evt, minlength=NEXP)
    kcap = -(-counts // NCORES)
    T = int(kcap.sum())
    order = np.argsort(evt, kind="stable")
    cum = np.concatenate([[0], np.cumsum(counts)])
    assign = np.full((NCORES, T), -1, dtype=np.int64)
    pos = 0
    segs = []  # (expert, sample_start_within_core, n_samples)
    for g in range(NEXP):
        kg = int(kcap[g])
        if kg == 0:
            continue
        Ig = order[cum[g] : cum[g + 1]]
        for c in range(NCORES):
            seg = Ig[c * kg : (c + 1) * kg]
            assign[c, pos : pos + len(seg)] = seg
        segs.append((g, pos, kg))
        pos += kg
    return dict(kcap=kcap, T=T, R=T * E, assign=assign, segs=segs)


def build_nc(plan, repeat=1, relu_dve_frac=0.7):
    T, R = plan["T"], plan["R"]
    BS = 2 * SC  # 36 samples per block
    BC = BS * E  # 1008 cols
    nc = bacc.Bacc("TRN2", target_bir_lowering=False, debug=False,
                   num_devices=NCORES)
    argt = nc.dram_tensor("argt", [DP, R], BF16, kind="ExternalInput").ap()
    evtt = nc.dram_tensor("evtt", [DP, T], BF16, kind="ExternalInput").ap()
    wma = nc.dram_tensor("wma", [DP, DP], BF16, kind="ExternalInput").ap()
    wme = nc.dram_tensor("wme", [DP, DP], BF16, kind="ExternalInput").ap()
    w1t = nc.dram_tensor("w1t", [DP, NEXP * H], BF16, kind="ExternalInput").ap()
    w2t = nc.dram_tensor("w2t", [H, NEXP * R_OUT], BF16, kind="ExternalInput").ap()
    bmt = nc.dram_tensor("bmt", [P, KT], F32, kind="ExternalInput").ap()
    b1t = nc.dram_tensor("b1t", [H, NEXP], F32, kind="ExternalInput").ap()
    b2t = nc.dram_tensor("b2t", [R_OUT, NEXP], F32, kind="ExternalInput").ap()
    out = nc.dram_tensor("out", [R_OUT, R], F32, kind="ExternalOutput").ap()

    nblocks = math.ceil(T / BS)
    relu_count = [0]

    with tile.TileContext(nc) as tc:
        with (
            tc.tile_pool(name="wpool", bufs=1) as wpool,
            tc.tile_pool(name="big", bufs=1) as big,
            tc.tile_pool(name="io", bufs=3) as io,
            tc.tile_pool(name="et", bufs=3) as etpool,
            tc.tile_pool(name="psm", bufs=2, space="PSUM") as psm,
            tc.tile_pool(name="pse", bufs=2, space="PSUM") as pse,
        ):
            wma_sb, wme_sb, w1_sb, evt_sb, yev_sb = [], [], [], [], []
            # yev-phase dependencies first so PE can start ASAP
            for k in range(KT):
                we = wpool.tile([P, DP], BF16, tag=f"wme{k}")
                nc.sync.dma_start(we[:], wme[k * P : (k + 1) * P, :])
                wme_sb.append(we)
                ev = wpool.tile([P, T], BF16, tag=f"evt{k}")
                nc.sync.dma_start(ev[:], evtt[k * P : (k + 1) * P, :])
                evt_sb.append(ev)
            bm_sb = wpool.tile([P, KT], F32, tag="bm")
            nc.sync.dma_start(bm_sb[:], bmt[:])
            for k in range(KT):
                wa = wpool.tile([P, DP], BF16, tag=f"wma{k}")
                nc.sync.dma_start(wa[:], wma[k * P : (k + 1) * P, :])
                wma_sb.append(wa)
            for k in range(KT):
                w1 = wpool.tile([P, NEXP * H], BF16, tag=f"w1{k}")
                nc.sync.dma_start(w1[:], w1t[k * P : (k + 1) * P, :])
                w1_sb.append(w1)
            w2_sb = wpool.tile([H, NEXP * R_OUT], BF16, tag="w2")
            nc.sync.dma_start(w2_sb[:], w2t[:])
            b1_sb = wpool.tile([H, NEXP], F32, tag="b1")
            nc.sync.dma_start(b1_sb[:], b1t[:])
            b2_sb = wpool.tile([R_OUT, NEXP], F32, tag="b2")
            nc.sync.dma_start(b2_sb[:], b2t[:])
            for m in range(KT):
                yv = wpool.tile([P, T], F32, tag=f"yev{m}")
                yev_sb.append(yv)
            merged = [
                big.tile([P, R], BF16, tag=f"merged{m}", name=f"merged{m}")
                for m in range(KT)
            ]

            def emit_expert_seg(g, g0, kg):
                c0, nn = g0 * E, kg * E
                qs = [(q0, min(504, nn - q0)) for q0 in range(0, nn, 504)]
                hp = pse.tile([P, 1024], F32, tag="pse", name="hp")
                for k in range(KT):
                    for qi, (q0, qn) in enumerate(qs):
                        nc.tensor.matmul(
                            hp[:H, qi * 512 : qi * 512 + qn],
                            w1_sb[k][:, g * H : (g + 1) * H],
                            merged[k][:, c0 + q0 : c0 + q0 + qn],
                            start=(k == 0),
                            stop=(k == KT - 1),
                        )
                ht = etpool.tile([H, BC], BF16, tag="htile")
                for qi, (q0, qn) in enumerate(qs):
                    nc.scalar.activation(
                        ht[:, q0 : q0 + qn], hp[:H, qi * 512 : qi * 512 + qn],
                        mybir.ActivationFunctionType.Relu,
                        bias=b1_sb[:, g : g + 1],
                    )
                op = pse.tile([P, 1024], F32, tag="pse", name="op")
                for qi, (q0, qn) in enumerate(qs):
                    nc.tensor.matmul(
                        op[:R_OUT, qi * 512 : qi * 512 + qn],
                        w2_sb[:, g * R_OUT : (g + 1) * R_OUT],
                        ht[:, q0 : q0 + qn],
                        start=True, stop=True,
                    )
                ot = etpool.tile([R_OUT, BC], F32, tag="otile")
                for qi, (q0, qn) in enumerate(qs):
                    nc.scalar.activation(
                        ot[:, q0 : q0 + qn], op[:R_OUT, qi * 512 : qi * 512 + qn],
                        mybir.ActivationFunctionType.Identity,
                        bias=b2_sb[:, g : g + 1],
                    )
                nc.sync.dma_start(out[:, c0 : c0 + nn], ot[:, :nn])

            for _rep in range(repeat):
                # yev = Wm_bot.T @ ev.T + bm (PE warmup too)
                for m in range(KT):
                    for c0 in range(0, T, 504):
                        nn = min(504, T - c0)
                        pst = psm.tile([P, 1024], F32, tag="psm", name="pst")
                        for k in range(KT):
                            nc.tensor.matmul(
                                pst[:, :nn],
                                wme_sb[k][:, m * P : (m + 1) * P],
                                evt_sb[k][:, c0 : c0 + nn],
                                start=(k == 0), stop=(k == KT - 1),
                            )
                        nc.scalar.activation(
                            yev_sb[m][:, c0 : c0 + nn], pst[:, :nn],
                            mybir.ActivationFunctionType.Identity,
                            bias=bm_sb[:, m : m + 1],
                        )

                emitted = [False] * len(plan["segs"])

                def emit_covered(s_done):
                    for i, (g, g0, kg) in enumerate(plan["segs"]):
                        if not emitted[i] and g0 + kg <= s_done:
                            emit_expert_seg(g, g0, kg)
                            emitted[i] = True

                for b in range(nblocks):
                    s0 = b * BS
                    ns_blk = min(BS, T - s0)
                    col0 = s0 * E
                    bcols = ns_blk * E
                    at = []
                    for k in range(KT):
                        a = io.tile([P, BC], BF16, tag=f"arg{k}")
                        nc.sync.dma_start(
                            a[:, :bcols],
                            argt[k * P : (k + 1) * P, col0 : col0 + bcols],
                        )
                        at.append(a)
                    qs = [(q0, min(504, bcols - q0)) for q0 in range(0, bcols, 504)]
                    for m in range(KT):
                        pst = psm.tile([P, 1024], F32, tag="psm", name="pst")
                        for k in range(KT):
                            for qi, (q0, qn) in enumerate(qs):
                                nc.tensor.matmul(
                                    pst[:, qi * 512 : qi * 512 + qn],
                                    wma_sb[k][:, m * P : (m + 1) * P],
                                    at[k][:, q0 : q0 + qn],
                                    start=(k == 0), stop=(k == KT - 1),
                                )
                        for qi, (q0, qn) in enumerate(qs):
                            qsmp = qn // E
                            dst = merged[m][:, col0 + q0 : col0 + q0 + qn]
                            pv = pst[:, qi * 512 : qi * 512 + qn]
                            ps3 = pv.rearrange("p (s e) -> p s e", e=E)
                            d3 = dst.rearrange("p (s e) -> p s e", e=E)
                            yv = yev_sb[m][:, s0 + qi * SC : s0 + qi * SC + qsmp]
                            yv3 = bass.AP(yv.tensor, yv.offset, list(yv.ap) + [[0, E]])
                            nc.vector.tensor_tensor(d3, ps3, yv3, mybir.AluOpType.add)
                            relu_count[0] += 1
                            on_dve = math.floor(
                                relu_count[0] * relu_dve_frac
                            ) != math.floor((relu_count[0] - 1) * relu_dve_frac)
                            if on_dve:
                                nc.vector.tensor_scalar_max(dst, dst, 0.0)
                            else:
                                nc.scalar.activation(
                                    dst, dst, mybir.ActivationFunctionType.Relu
                                )
                    if b >= 1:
                        emit_covered(b * BS)
                emit_covered(T)

    nc.compile()
    return nc


def make_in_maps(inputs, plan):
    arg = np.asarray(inputs["arg_mention_embeds"], dtype=np.float32)
    ev = np.asarray(inputs["event_mention_embed"], dtype=np.float32)
    Wm = np.asarray(inputs["Wm"], dtype=np.float32)
    bm = np.asarray(inputs["bm"], dtype=np.float32)
    W1 = np.asarray(inputs["W1"], dtype=np.float32)
    b1 = np.asarray(inputs["b1"], dtype=np.float32)
    W2 = np.asarray(inputs["W2"], dtype=np.float32)
    b2 = np.asarray(inputs["b2"], dtype=np.float32)
    T, R = plan["T"], plan["R"]
    assign = plan["assign"]

    wma_np = np.zeros((DP, DP), NPBF16)
    wma_np[:D, :M] = Wm[:D].astype(NPBF16)
    wme_np = np.zeros((DP, DP), NPBF16)
    wme_np[:D, :M] = Wm[D:].astype(NPBF16)
    w1t_np = np.zeros((DP, NEXP * H), NPBF16)
    w1t_np[:M] = W1.transpose(1, 0, 2).reshape(M, NEXP * H).astype(NPBF16)
    w2t_np = np.ascontiguousarray(
        W2.transpose(1, 0, 2).reshape(H, NEXP * R_OUT).astype(NPBF16)
    )
    bm_pad = np.zeros(DP, np.float32)
    bm_pad[:M] = bm
    bmt_np = np.ascontiguousarray(bm_pad.reshape(KT, P).T)
    b1t_np = np.ascontiguousarray(b1.T.astype(np.float32))
    b2t_np = np.ascontiguousarray(b2.T.astype(np.float32))

    in_maps = []
    for c in range(NCORES):
        idx = assign[c]
        mask = idx >= 0
        ac = np.zeros((T, E, D), np.float32)
        ac[mask] = arg[idx[mask]]
        evc = np.zeros((T, D), np.float32)
        evc[mask] = ev[idx[mask], 0]
        argt_np = np.zeros((DP, R), NPBF16)
        argt_np[:D] = ac.reshape(T * E, D).T.astype(NPBF16)
        evt_np = np.zeros((DP, T), NPBF16)
        evt_np[:D] = evc.T.astype(NPBF16)
        in_maps.append(
            dict(
                argt=argt_np, evtt=evt_np, wma=wma_np, wme=wme_np,
                w1t=w1t_np, w2t=w2t_np, bmt=bmt_np, b1t=b1t_np, b2t=b2t_np,
            )
        )
    return in_maps


def assemble_output(results, plan):
    T = plan["T"]
    assign = plan["assign"]
    res = np.zeros((B, E, R_OUT), np.float32)
    for c in range(NCORES):
        oc = np.asarray(results[c]["out"])  # [16, R]
        oc = oc.reshape(R_OUT, T, E).transpose(1, 2, 0)
        idx = assign[c]
        mask = idx >= 0
        res[idx[mask]] = oc[mask]
    return res


def kernel(**inputs) -> np.ndarray:
    plan = plan_from_evt(inputs["evt_type_list"])
    nc = build_nc(plan)
    in_maps = make_in_maps(inputs, plan)
    res = run_bass_kernel_spmd(nc, in_maps, core_ids=list(range(NCORES)))
    LAST_INFO["plan"] = plan
    LAST_INFO["exec_time_ns"] = res.exec_time_ns
    return assemble_output(res.results, plan)


# revision 6
# speedup vs baseline: 3.3023x; 3.3023x over previous
"""MoE arg-classifier (nn_ArgClassifierLayer) on 8 Trainium2 NeuronCores.

Strategy (pure data-parallel over batch, no collectives):
  - The host sorts samples by expert id and deals them to the 8 cores so
    every core holds an identical per-expert segment layout (capacity
    ceil(count/8) per expert, zero-padded). Segment boundaries become
    compile-time constants, so the per-sample grouped GEMM turns into a few
    dense GEMMs over contiguous column ranges.
  - All activations live feature-major ([feat, row]) on chip; the host
    pre-transposes inputs and post-transposes the output, so the device
    never transposes anything.
  - concat([arg, ev]) @ Wm is split into arg @ Wm_top (per entity row) +
    ev @ Wm_bot (once per sample, broadcast over the 28 entity rows with a
    stride-0 access pattern), halving the dominant GEMM's FLOPs.
  - bf16 matmul inputs with fp32 PSUM accumulation; fp32 biases/output.
  - Expert layers are software-pipelined (L1 of segment i+1 emitted between
    L1(i) and L2(i)) and ride one block behind the merge GEMM so the
    TensorEngine never waits on activation drains.
"""

import contextlib
import math

import numpy as np
import ml_dtypes

import concourse.bass as bass
import concourse.tile as tile
from concourse import bacc, mybir
from concourse.bass_utils import run_bass_kernel_spmd

BF16 = mybir.dt.bfloat16
F32 = mybir.dt.float32
NPBF16 = ml_dtypes.bfloat16

B, E, D = 4096, 28, 500
M, H, R_OUT = 500, 64, 16
NEXP = 34
NCORES = 8
DP = 512  # padded feature dim (both D and M pad to 512)
KT = 4  # k-tiles of 128
P = 128
SC = 18  # samples per compute chunk -> 504 cols <= one PSUM bank
CC = SC * E  # 504

LAST_INFO = {}


def plan_from_evt(evt):
    evt = np.asarray(evt).astype(np.int64)
    counts = np.bincount(evt, minlength=NEXP)
    kcap = -(-counts // NCORES)
    T = int(kcap.sum())
    order = np.argsort(evt, kind="stable")
    cum = np.concatenate([[0], np.cumsum(counts)])
    assign = np.full((NCORES, T), -1, dtype=np.int64)
    pos = 0
    segs = []  # (expert, sample_start_within_core, n_samples)
    for g in range(NEXP):
        kg = int(kcap[g])
        if kg == 0:
            continue
        Ig = order[cum[g] : cum[g + 1]]
        for c in range(NCORES):
            seg = Ig[c * kg : (c + 1) * kg]
            assign[c, pos : pos + len(seg)] = seg
        segs.append((g, pos, kg))
        pos += kg
    return dict(kcap=kcap, T=T, R=T * E, assign=assign, segs=segs)


def build_nc(plan, relu_dve_frac=0.0, loop_n=None, hint=True, repeat=1):
    T, R = plan["T"], plan["R"]
    BS = 2 * SC  # 36 samples per block
    BC = BS * E  # 1008 cols
    nc = bacc.Bacc("TRN2", target_bir_lowering=False, debug=False,
                   num_devices=NCORES)
    argt = nc.dram_tensor("argt", [DP, R], BF16, kind="ExternalInput").ap()
    evtt = nc.dram_tensor("evtt", [DP, T], BF16, kind="ExternalInput").ap()
    wma = nc.dram_tensor("wma", [DP, DP], BF16, kind="ExternalInput").ap()
    wme = nc.dram_tensor("wme", [DP, DP], BF16, kind="ExternalInput").ap()
    w1t = nc.dram_tensor("w1t", [DP, NEXP * H], BF16, kind="ExternalInput").ap()
    w2t = nc.dram_tensor("w2t", [H, NEXP * R_OUT], BF16, kind="ExternalInput").ap()
    bmt = nc.dram_tensor("bmt", [P, KT], F32, kind="ExternalInput").ap()
    b1t = nc.dram_tensor("b1t", [H, NEXP], F32, kind="ExternalInput").ap()
    b2t = nc.dram_tensor("b2t", [R_OUT, NEXP], F32, kind="ExternalInput").ap()
    out = nc.dram_tensor("out", [R_OUT, R], F32, kind="ExternalOutput").ap()

    nblocks = math.ceil(T / BS)
    relu_count = [0]

    with tile.TileContext(nc) as tc:
        with (
            tc.tile_pool(name="wpool", bufs=1) as wpool,
            tc.tile_pool(name="big", bufs=1) as big,
            tc.tile_pool(name="io", bufs=3) as io,
            tc.tile_pool(name="et", bufs=3) as etpool,
            tc.tile_pool(name="psm", bufs=4, space="PSUM") as psm,
            tc.tile_pool(name="pse", bufs=2, space="PSUM") as pse,
        ):
            wma_sb, wme_sb, w1_sb, evt_sb, yev_sb = [], [], [], [], []
            # yev-phase dependencies first so PE can start ASAP
            for k in range(KT):
                we = wpool.tile([P, DP], BF16, tag=f"wme{k}")
                nc.sync.dma_start(we[:], wme[k * P : (k + 1) * P, :])
                wme_sb.append(we)
                ev = wpool.tile([P, T], BF16, tag=f"evt{k}")
                nc.sync.dma_start(ev[:], evtt[k * P : (k + 1) * P, :])
                evt_sb.append(ev)
            bm_sb = wpool.tile([P, KT], F32, tag="bm")
            nc.sync.dma_start(bm_sb[:], bmt[:])
            for k in range(KT):
                wa = wpool.tile([P, DP], BF16, tag=f"wma{k}")
                nc.sync.dma_start(wa[:], wma[k * P : (k + 1) * P, :])
                wma_sb.append(wa)
            for k in range(KT):
                w1 = wpool.tile([P, NEXP * H], BF16, tag=f"w1{k}")
                nc.sync.dma_start(w1[:], w1t[k * P : (k + 1) * P, :])
                w1_sb.append(w1)
            w2_sb = wpool.tile([H, NEXP * R_OUT], BF16, tag="w2")
            nc.sync.dma_start(w2_sb[:], w2t[:])
            b1_sb = wpool.tile([H, NEXP], F32, tag="b1")
            nc.sync.dma_start(b1_sb[:], b1t[:])
            b2_sb = wpool.tile([R_OUT, NEXP], F32, tag="b2")
            nc.sync.dma_start(b2_sb[:], b2t[:])
            for m in range(KT):
                yv = wpool.tile([P, T], F32, tag=f"yev{m}")
                yev_sb.append(yv)
            merged = [
                big.tile([P, R], BF16, tag=f"merged{m}", name=f"merged{m}")
                for m in range(KT)
            ]

            pending = []  # expert segments between L1 and L2 emission

            def emit_l1(g, g0, kg):
                c0, nn = g0 * E, kg * E
                qs = [(q0, min(504, nn - q0)) for q0 in range(0, nn, 504)]
                hp = pse.tile([P, 1024], F32, tag="pse", name="hp")
                for k in range(KT):
                    for qi, (q0, qn) in enumerate(qs):
                        nc.tensor.matmul(
                            hp[:H, qi * 512 : qi * 512 + qn],
                            w1_sb[k][:, g * H : (g + 1) * H],
                            merged[k][:, c0 + q0 : c0 + q0 + qn],
                            start=(k == 0), stop=(k == KT - 1),
                        )
                ht = etpool.tile([H, 1024], BF16, tag="htile")
                for qi, (q0, qn) in enumerate(qs):
                    nc.scalar.activation(
                        ht[:, q0 : q0 + qn], hp[:H, qi * 512 : qi * 512 + qn],
                        mybir.ActivationFunctionType.Relu,
                        bias=b1_sb[:, g : g + 1],
                    )
                pending.append((g, c0, nn, qs, ht))

            def emit_l2():
                g, c0, nn, qs, ht = pending.pop(0)
                op = pse.tile([P, 1024], F32, tag="pse", name="op")
                for qi, (q0, qn) in enumerate(qs):
                    nc.tensor.matmul(
                        op[:R_OUT, qi * 512 : qi * 512 + qn],
                        w2_sb[:, g * R_OUT : (g + 1) * R_OUT],
                        ht[:, q0 : q0 + qn],
                        start=True, stop=True,
                    )
                ot = etpool.tile([R_OUT, 1024], F32, tag="otile")
                for qi, (q0, qn) in enumerate(qs):
                    nc.scalar.activation(
                        ot[:, q0 : q0 + qn], op[:R_OUT, qi * 512 : qi * 512 + qn],
                        mybir.ActivationFunctionType.Identity,
                        bias=b2_sb[:, g : g + 1],
                    )
                nc.sync.dma_start(out[:, c0 : c0 + nn], ot[:, :nn])

            def body_ctx():
                if loop_n is not None:
                    hints = (
                        (mybir.EngineType.PE, mybir.EngineType.Activation,
                         mybir.EngineType.DVE, mybir.EngineType.SP)
                        if hint else ()
                    )
                    return tc.For_i(0, loop_n, 1, hint_engines=hints)
                return contextlib.nullcontext(0)

            def emit_body():
                # yev = Wm_bot.T @ ev.T + bm (PE warmup too)
                for m in range(KT):
                    for c0 in range(0, T, 504):
                        nn = min(504, T - c0)
                        pst = psm.tile([P, 512], F32, tag="psm", name="pst")
                        for k in range(KT):
                            nc.tensor.matmul(
                                pst[:, :nn],
                                wme_sb[k][:, m * P : (m + 1) * P],
                                evt_sb[k][:, c0 : c0 + nn],
                                start=(k == 0), stop=(k == KT - 1),
                            )
                        nc.scalar.activation(
                            yev_sb[m][:, c0 : c0 + nn], pst[:, :nn],
                            mybir.ActivationFunctionType.Identity,
                            bias=bm_sb[:, m : m + 1],
                        )

                emitted = [False] * len(plan["segs"])

                def emit_covered(s_done):
                    for i, (g, g0, kg) in enumerate(plan["segs"]):
                        if not emitted[i] and g0 + kg <= s_done:
                            # split long segments so psum/sbuf tiles fit;
                            # L2 of the previous piece lands after L1 of the
                            # next so PE never waits on the relu drain
                            for p0 in range(0, kg, 2 * SC):
                                emit_l1(g, g0 + p0, min(2 * SC, kg - p0))
                                while len(pending) > 1:
                                    emit_l2()
                            emitted[i] = True

                for b in range(nblocks):
                    s0 = b * BS
                    ns_blk = min(BS, T - s0)
                    col0 = s0 * E
                    bcols = ns_blk * E
                    at = []
                    for k in range(KT):
                        a = io.tile([P, BC], BF16, tag=f"arg{k}")
                        nc.sync.dma_start(
                            a[:, :bcols],
                            argt[k * P : (k + 1) * P, col0 : col0 + bcols],
                        )
                        at.append(a)
                    qs = [(q0, min(504, bcols - q0)) for q0 in range(0, bcols, 504)]
                    for m in range(KT):
                        psts = [
                            psm.tile([P, 512], F32, tag="psm", name="pst")
                            for _ in qs
                        ]
                        for k in range(KT):
                            for qi, (q0, qn) in enumerate(qs):
                                nc.tensor.matmul(
                                    psts[qi][:, :qn],
                                    wma_sb[k][:, m * P : (m + 1) * P],
                                    at[k][:, q0 : q0 + qn],
                                    start=(k == 0), stop=(k == KT - 1),
                                )
                        for qi, (q0, qn) in enumerate(qs):
                            qsmp = qn // E
                            dst = merged[m][:, col0 + q0 : col0 + q0 + qn]
                            pv = psts[qi][:, :qn]
                            ps3 = pv.rearrange("p (s e) -> p s e", e=E)
                            d3 = dst.rearrange("p (s e) -> p s e", e=E)
                            yv = yev_sb[m][:, s0 + qi * SC : s0 + qi * SC + qsmp]
                            yv3 = bass.AP(yv.tensor, yv.offset, list(yv.ap) + [[0, E]])
                            nc.vector.tensor_tensor(d3, ps3, yv3, mybir.AluOpType.add)
                            relu_count[0] += 1
                            on_dve = math.floor(
                                relu_count[0] * relu_dve_frac
                            ) != math.floor((relu_count[0] - 1) * relu_dve_frac)
                            if on_dve:
                                nc.vector.tensor_scalar_max(dst, dst, 0.0)
                            else:
                                nc.scalar.activation(
                                    dst, dst, mybir.ActivationFunctionType.Relu
                                )
                    if b >= 1:
                        emit_covered(b * BS)
                emit_covered(T)
                while pending:
                    emit_l2()

            with body_ctx():
                for _rep in range(repeat):
                    emit_body()

    nc.compile()
    return nc


def make_in_maps(inputs, plan):
    arg = np.asarray(inputs["arg_mention_embeds"], dtype=np.float32)
    ev = np.asarray(inputs["event_mention_embed"], dtype=np.float32)
    Wm = np.asarray(inputs["Wm"], dtype=np.float32)
    bm = np.asarray(inputs["bm"], dtype=np.float32)
    W1 = np.asarray(inputs["W1"], dtype=np.float32)
    b1 = np.asarray(inputs["b1"], dtype=np.float32)
    W2 = np.asarray(inputs["W2"], dtype=np.float32)
    b2 = np.asarray(inputs["b2"], dtype=np.float32)
    T, R = plan["T"], plan["R"]
    assign = plan["assign"]

    wma_np = np.zeros((DP, DP), NPBF16)
    wma_np[:D, :M] = Wm[:D].astype(NPBF16)
    wme_np = np.zeros((DP, DP), NPBF16)
    wme_np[:D, :M] = Wm[D:].astype(NPBF16)
    w1t_np = np.zeros((DP, NEXP * H), NPBF16)
    w1t_np[:M] = W1.transpose(1, 0, 2).reshape(M, NEXP * H).astype(NPBF16)
    w2t_np = np.ascontiguousarray(
        W2.transpose(1, 0, 2).reshape(H, NEXP * R_OUT).astype(NPBF16)
    )
    bm_pad = np.zeros(DP, np.float32)
    bm_pad[:M] = bm
    bmt_np = np.ascontiguousarray(bm_pad.reshape(KT, P).T)
    b1t_np = np.ascontiguousarray(b1.T.astype(np.float32))
    b2t_np = np.ascontiguousarray(b2.T.astype(np.float32))

    in_maps = []
    for c in range(NCORES):
        idx = assign[c]
        mask = idx >= 0
        ac = np.zeros((T, E, D), np.float32)
        ac[mask] = arg[idx[mask]]
        evc = np.zeros((T, D), np.float32)
        evc[mask] = ev[idx[mask], 0]
        argt_np = np.zeros((DP, R), NPBF16)
        argt_np[:D] = ac.reshape(T * E, D).T.astype(NPBF16)
        evt_np = np.zeros((DP, T), NPBF16)
        evt_np[:D] = evc.T.astype(NPBF16)
        in_maps.append(
            dict(
                argt=argt_np, evtt=evt_np, wma=wma_np, wme=wme_np,
                w1t=w1t_np, w2t=w2t_np, bmt=bmt_np, b1t=b1t_np, b2t=b2t_np,
            )
        )
    return in_maps


def assemble_output(results, plan):
    T = plan["T"]
    assign = plan["assign"]
    res = np.zeros((B, E, R_OUT), np.float32)
    for c in range(NCORES):
        oc = np.asarray(results[c]["out"])  # [16, R]
        oc = oc.reshape(R_OUT, T, E).transpose(1, 2, 0)
        idx = assign[c]
        mask = idx >= 0
        res[idx[mask]] = oc[mask]
    return res


def kernel(**inputs) -> np.ndarray:
    plan = plan_from_evt(inputs["evt_type_list"])
    nc = build_nc(plan)
    in_maps = make_in_maps(inputs, plan)
    res = run_bass_kernel_spmd(nc, in_maps, core_ids=list(range(NCORES)))
    LAST_INFO["plan"] = plan
    LAST_INFO["exec_time_ns"] = res.exec_time_ns
    return assemble_output(res.results, plan)
